# revision 21
# baseline (speedup 1.0000x reference)
"""Trainium2 Bass kernel for EnhancedSeq2Seq (2-layer LSTM enc/dec + attention + 2-expert top-1 MoE vocab head).

Sharding: batch-parallel recurrent part (64/8 = 8 rows per core),
vocab-parallel MoE head (32000/8 = 4000 per core). Token features are
all-gathered across cores every 2 decoder steps and the MoE projection is
pipelined behind the decoder recurrence.

Scale conventions inside the device program:
  - h state tiles hold H = 2*h ("doubled h") so the sigmoid can be computed
    as a single tanh: sigmoid(x) = 0.5 + 0.5*tanh(x/2).  All weights that
    consume h (or doubled context CTX2 = 2*ctx) are pre-halved on the host.
  - encoutT holds doubled encoder outputs, att_WT is pre-halved.
  - MoE expert blend (top-1, K=1 => gate weight == 1):
      out = xf@W1 + (m*xf)@(W0-W1) + b1 + m*(b0-b1),  m = 1 if expert0 wins.
"""

import os
import sys

sys.path.insert(0, "/opt/trn_rl_repo")

import numpy as np

import concourse.bass as bass
import concourse.mybir as mybir
import concourse.tile as tile
from concourse import bacc
from concourse.bass import IndirectOffsetOnAxis
from concourse.bass_utils import run_bass_kernel_spmd
from concourse.masks import make_identity

V, E, H = 32000, 64, 128
B, S, T = 64, 30, 20
NCORES = 8
BL = B // NCORES        # 8   local batch rows
VS = V // NCORES        # 4000 vocab shard
G4 = 4 * H              # 512
NTE = BL * S            # 240  encoder tokens / core
NTD = BL * T            # 160  decoder tokens / core
GSTEP = 2               # decoder steps per all-gather block
NBLK = T // GSTEP       # 10
TOKB = GSTEP * B        # 128  tokens per MoE block (all cores)
PAYR = 2 * H + 2        # 258  payload rows per step (H1, CTX2, ones, m)
VTILES = [(i * 512, min((i + 1) * 512, VS)) for i in range((VS + 511) // 512)]

f32 = mybir.dt.float32
f32r = mybir.dt.float32r
i32 = mybir.dt.int32
AF = mybir.ActivationFunctionType
ALU = mybir.AluOpType
AX = mybir.AxisListType

_cache = {}

# debug toggles for bisection
DBG_COLLECTIVE = True
DBG_PBCAST = True
DBG_GATHER = True
DBG_PHASE = 4  # 1=setup/embed/ihpre 2=+encoder 3=+decoder 4=+moe/collective


def _build_program():
    nc = bacc.Bacc("TRN2", target_bir_lowering=False, debug=False, num_devices=NCORES)

    # ---------------- I/O -------------------------------------------------
    din = {}

    def dram_in(name, shape, dtype=f32):
        din[name] = nc.dram_tensor(name, list(shape), dtype, kind="ExternalInput")
        return din[name]

    src_idx = dram_in("src_idx", [2, NTE // 2, 1], i32)
    trg_idx = dram_in("trg_idx", [2, NTD // 2, 1], i32)
    emb = dram_in("emb", [V, E])
    wih0T = dram_in("wih0T", [E, G4])
    whh0T = dram_in("whh0T", [H, G4])
    b0g = dram_in("b0g", [H, 4])
    wih1T = dram_in("wih1T", [H, G4])
    whh1T = dram_in("whh1T", [H, G4])
    b1g = dram_in("b1g", [H, 4])
    dwih0xT = dram_in("dwih0xT", [E, G4])
    dwih0cT = dram_in("dwih0cT", [H, G4])
    dwhh0T = dram_in("dwhh0T", [H, G4])
    db0g = dram_in("db0g", [H, 4])
    dwih1T = dram_in("dwih1T", [H, G4])
    dwhh1T = dram_in("dwhh1T", [H, G4])
    db1g = dram_in("db1g", [H, 4])
    attWT = dram_in("attWT", [H, H])
    attb = dram_in("attb", [H, 1])
    attv = dram_in("attv", [H, 1])
    wd12 = dram_in("wd12", [H, 2])
    gdb = dram_in("gdb", [1, 1])
    w1a = dram_in("w1a", [H, VS], f32r)
    w1b = dram_in("w1b", [H, VS], f32r)
    wda = dram_in("wda", [H, VS], f32r)
    wdb = dram_in("wdb", [H, VS], f32r)
    bias2 = dram_in("bias2", [2, VS], f32r)

    out = nc.dram_tensor("out", [NBLK * TOKB, VS], f32, kind="ExternalOutput")

    with tile.TileContext(nc) as tc:
        with (
            tc.tile_pool(name="wc", bufs=1) as wc,            # constants / persistents
            tc.tile_pool(name="sb", bufs=2) as sb,            # rotating work tiles
            tc.tile_pool(name="sb3", bufs=3) as sb3,          # recurrent state tiles
            tc.tile_pool(name="sbm", bufs=2) as sbm,          # MoE activation tiles
            tc.tile_pool(name="sbo", bufs=4) as sbo,          # MoE output staging
            tc.tile_pool(name="ppc", bufs=2, space="PSUM") as ppc,   # cell psums
            tc.tile_pool(name="ppe", bufs=2, space="PSUM") as ppe,   # [128,240] psums
            tc.tile_pool(name="pps", bufs=2, space="PSUM") as pps,   # small psums
            tc.tile_pool(name="ppo", bufs=2, space="PSUM") as ppo,   # MoE out psums
            tc.tile_pool(name="dr", bufs=3, space="DRAM") as dr,     # collective bufs
        ):
            # ---------------- constant loads ------------------------------
            def const_tile(name, shape, dtype=f32):
                t = wc.tile(list(shape), dtype, tag=name, name=name)
                nc.sync.dma_start(out=t[:], in_=din[name][:])
                return t

            c_wih0T = const_tile("wih0T", [E, G4])
            c_whh0T = const_tile("whh0T", [H, G4])
            c_b0g = const_tile("b0g", [H, 4])
            c_wih1T = const_tile("wih1T", [H, G4])
            c_whh1T = const_tile("whh1T", [H, G4])
            c_b1g = const_tile("b1g", [H, 4])
            c_dwih0xT = const_tile("dwih0xT", [E, G4])
            c_dwih0cT = const_tile("dwih0cT", [H, G4])
            c_dwhh0T = const_tile("dwhh0T", [H, G4])
            c_db0g = const_tile("db0g", [H, 4])
            c_dwih1T = const_tile("dwih1T", [H, G4])
            c_dwhh1T = const_tile("dwhh1T", [H, G4])
            c_db1g = const_tile("db1g", [H, 4])
            c_attWT = const_tile("attWT", [H, H])
            c_attb = const_tile("attb", [H, 1])
            c_attv = const_tile("attv", [H, 1])
            c_wd12 = const_tile("wd12", [H, 2])
            c_gdb = const_tile("gdb", [1, 1])
            # big MoE weights (emitted after the small ones)
            c_w1a = const_tile("w1a", [H, VS], f32r)
            c_w1b = const_tile("w1b", [H, VS], f32r)
            c_wda = const_tile("wda", [H, VS], f32r)
            c_wdb = const_tile("wdb", [H, VS], f32r)
            c_bias2 = const_tile("bias2", [2, VS], f32r)

            idt = wc.tile([H, H], f32, tag="idt", name="idt")
            make_identity(nc, idt[:])

            # persistent activations
            ihpre0 = wc.tile([H, S * 4 * BL], f32, tag="ihpre0", name="ihpre0")
            decihp = wc.tile([H, T * 4 * BL], f32, tag="decihp", name="decihp")
            encoutT = wc.tile([H, NTE], f32, tag="encoutT", name="encoutT")   # (b, s) cols
            encprojT = wc.tile([H, NTE], f32, tag="encprojT", name="encprojT")
            ones_row = wc.tile([1, BL], f32, tag="ones_row", name="ones_row")
            nc.vector.memset(ones_row[:], 1.0)
            ones_l = wc.tile([1, H], f32, tag="ones_l", name="ones_l")
            nc.vector.memset(ones_l[:], 1.0)

            def pbcast(dst, src_row, n):
                if DBG_PBCAST:
                    nc.gpsimd.partition_broadcast(dst[:], src_row)
                else:
                    pb = ppe.tile([H, n], f32, tag="pih")
                    nc.tensor.matmul(pb[:], lhsT=ones_l[:], rhs=src_row, start=True, stop=True)
                    nc.scalar.copy(out=dst[:], in_=pb[:])

            # ---------------- embedding gather + transpose ----------------
            def embed(idx_dram, nchunks, chunk, xT):
                for i in range(nchunks):
                    isb = sb.tile([chunk, 1], i32, tag="isb")
                    nc.sync.dma_start(out=isb[:], in_=idx_dram[i])
                    gat = sb.tile([chunk, E], f32, tag="embg")
                    if DBG_GATHER:
                        nc.gpsimd.indirect_dma_start(
                            out=gat[:],
                            out_offset=None,
                            in_=emb[:, :],
                            in_offset=IndirectOffsetOnAxis(ap=isb[:, 0:1], axis=0),
                        )
                    else:
                        nc.sync.dma_start(out=gat[:], in_=emb[0:chunk, :])
                    pst = pps.tile([E, chunk], f32, tag="ps")
                    nc.tensor.transpose(pst[:], gat[:], idt[0:chunk, 0:chunk])
                    nc.scalar.copy(out=xT[:, i * chunk : (i + 1) * chunk], in_=pst[:])

            xT = wc.tile([E, NTE], f32, tag="xT", name="xT")
            xdT = wc.tile([E, NTD], f32, tag="xdT", name="xdT")
            embed(src_idx, 2, NTE // 2, xT)
            embed(trg_idx, 2, NTD // 2, xdT)

            # ---------------- ih precomputes ------------------------------
            def ih_pre(dst, nt, wT, rhs, bg):
                dview = dst[:].rearrange("p (t g b) -> p t g b", t=nt, g=4, b=BL)
                for g in range(4):
                    ps = ppe.tile([H, nt * BL], f32, tag="pih")
                    nc.tensor.matmul(
                        ps[:], lhsT=wT[:, g * H : (g + 1) * H], rhs=rhs[:],
                        start=True, stop=True,
                    )
                    nc.scalar.activation(
                        out=dview[:, :, g, :],
                        in_=ps[:].rearrange("p (t b) -> p t b", t=nt, b=BL),
                        func=AF.Identity,
                        bias=bg[:, g : g + 1],
                    )

            ih_pre(ihpre0, S, c_wih0T, xT, c_b0g)
            ih_pre(decihp, T, c_dwih0xT, xdT, c_db0g)

            # ---------------- LSTM cell helper ----------------------------
            def bias_bcast(bg):
                return bg[:].unsqueeze(2).to_broadcast([H, 4, BL])

            def lstm_cell(tag, mats, z_pre_ap, z_bias, c_prev, h_out_ap):
                """mats: list of (lhsT_full[H,512], rhs_ap, start) matmul ops.
                z_pre_ap: SBUF precomputed (ih+bias) slice to add, or None.
                z_bias:   [H,4] bias tile to broadcast-add, or None.
                Returns new c tile. Writes H (=2h) into h_out_ap."""
                if mats:
                    ps = ppc.tile([H, 4 * BL], f32, tag="pz")
                    for g in range(4):
                        for j, (lhsT, rhs) in enumerate(mats):
                            nc.tensor.matmul(
                                ps[:, g * BL : (g + 1) * BL],
                                lhsT=lhsT[:, g * H : (g + 1) * H],
                                rhs=rhs,
                                start=(j == 0),
                                stop=(j == len(mats) - 1),
                            )
                    z = sb.tile([H, 4 * BL], f32, tag="z_" + tag)
                    if z_pre_ap is not None:
                        nc.vector.tensor_add(out=z[:], in0=ps[:], in1=z_pre_ap)
                    else:
                        nc.vector.tensor_tensor(
                            out=z[:].rearrange("p (g b) -> p g b", g=4),
                            in0=ps[:].rearrange("p (g b) -> p g b", g=4),
                            in1=bias_bcast(z_bias),
                            op=ALU.add,
                        )
                    z_ap = z[:]
                else:
                    z_ap = z_pre_ap
                tio = sb.tile([H, 3 * BL], f32, tag="tio_" + tag)
                nc.scalar.activation(out=tio[:], in_=z_ap[:, 0 : 3 * BL], func=AF.Tanh, scale=0.5)
                tg = sb.tile([H, BL], f32, tag="tg_" + tag)
                nc.scalar.activation(out=tg[:], in_=z_ap[:, 3 * BL : 4 * BL], func=AF.Tanh)
                # cS carries 2*c ("doubled c"): avoids a separate 0.5 scale op
                bb = sb.tile([H, BL], f32, tag="bb_" + tag)
                nc.vector.scalar_tensor_tensor(
                    out=bb[:], in0=tio[:, 0:BL], scalar=1.0, in1=tg[:],
                    op0=ALU.add, op1=ALU.mult,
                )
                cS = sb3.tile([H, BL], f32, tag="c_" + tag)
                if c_prev is None:
                    nc.vector.tensor_copy(out=cS[:], in_=bb[:])
                else:
                    aa = sb.tile([H, BL], f32, tag="aa_" + tag)
                    nc.vector.scalar_tensor_tensor(
                        out=aa[:], in0=tio[:, BL : 2 * BL], scalar=1.0, in1=c_prev,
                        op0=ALU.add, op1=ALU.mult,
                    )
                    nc.vector.scalar_tensor_tensor(
                        out=cS[:], in0=aa[:], scalar=0.5, in1=bb[:],
                        op0=ALU.mult, op1=ALU.add,
                    )
                tch = sb.tile([H, BL], f32, tag="tc_" + tag)
                nc.scalar.activation(out=tch[:], in_=cS[:], func=AF.Tanh, scale=0.5)
                nc.vector.scalar_tensor_tensor(
                    out=h_out_ap, in0=tio[:, 2 * BL : 3 * BL], scalar=1.0, in1=tch[:],
                    op0=ALU.add, op1=ALU.mult,
                )
                return cS

            # ---------------- encoder ------------------------------------
            enc_view = encoutT[:].rearrange("p (b s) -> p b s", b=BL, s=S)
            if DBG_PHASE < 2:
                S_eff = 0
            else:
                S_eff = S
            h0 = c0 = c1 = None
            h1_ap = None
            for t in range(S_eff):
                mats0 = [] if t == 0 else [(c_whh0T, h0[:])]
                h0n = sb3.tile([H, BL], f32, tag="h0e")
                c0 = lstm_cell(
                    "e0", mats0, ihpre0[:, t * 4 * BL : (t + 1) * 4 * BL],
                    None, c0 if c0 is None else c0[:], h0n[:],
                )
                h0 = h0n
                mats1 = [(c_wih1T, h0[:])]
                if t > 0:
                    mats1.append((c_whh1T, h1_ap))
                h1_ap = enc_view[:, :, t]
                c1 = lstm_cell("e1", mats1, None, c_b1g, c1 if c1 is None else c1[:], h1_ap)

            # ---------------- encoder projection --------------------------
            run_dec = DBG_PHASE >= 3 and S_eff == S
            if S_eff == S:
                psP = ppe.tile([H, NTE], f32, tag="pih")
                nc.tensor.matmul(psP[:], lhsT=c_attWT[:], rhs=encoutT[:], start=True, stop=True)
                nc.scalar.activation(
                    out=encprojT[:], in_=psP[:], func=AF.Identity, bias=c_attb[:, 0:1]
                )

            # ---------------- decoder + MoE -------------------------------
            def moe_block(blk, gat):
                xf1 = sbm.tile([H, TOKB], f32r, tag="xf1")
                xf2 = sbm.tile([H, TOKB], f32r, tag="xf2")
                b2T = sbm.tile([2, TOKB], f32r, tag="b2T")
                for dst, r0, r1 in ((xf1, 0, H), (xf2, H, 2 * H), (b2T, 2 * H, PAYR)):
                    nc.sync.dma_start(
                        out=dst[:].rearrange(
                            "p (c s b) -> p c s b", c=NCORES, s=GSTEP, b=BL
                        ),
                        in_=gat[:, :, r0:r1, :].rearrange("c s r b -> r c s b").bitcast(f32r),
                    )
                mRow = sbm.tile([1, TOKB], f32, tag="mRow")
                nc.sync.dma_start(
                    out=mRow[:].rearrange("p (c s b) -> p c s b", c=NCORES, s=GSTEP, b=BL),
                    in_=gat[:, :, PAYR - 1 : PAYR, :].rearrange("c s r b -> r c s b"),
                )
                mB = sbm.tile([H, TOKB], f32, tag="mB")
                pbcast(mB, mRow[:], TOKB)
                x01 = sbm.tile([H, TOKB], f32r, tag="x01")
                x02 = sbm.tile([H, TOKB], f32r, tag="x02")
                nc.vector.tensor_mul(out=x01[:], in0=xf1[:], in1=mB[:])
                nc.vector.tensor_mul(out=x02[:], in0=xf2[:], in1=mB[:])
                for lo, hi in VTILES:
                    w = hi - lo
                    po = ppo.tile([TOKB, 512], f32, tag="po")
                    sl = slice(lo, hi)
                    mms = [
                        (xf1, c_w1a), (xf2, c_w1b), (x01, c_wda), (x02, c_wdb), (b2T, c_bias2),
                    ]
                    for j, (lt, rt) in enumerate(mms):
                        nc.tensor.matmul(
                            po[:, 0:w],
                            lhsT=lt[:],
                            rhs=rt[:, sl],
                            start=(j == 0),
                            stop=(j == len(mms) - 1),
                        )
                    st = sbo.tile([TOKB, 512], f32, tag="st")
                    nc.scalar.copy(out=st[:, 0:w], in_=po[:, 0:w])
                    nc.sync.dma_start(
                        out=out[blk * TOKB : (blk + 1) * TOKB, sl], in_=st[:, 0:w]
                    )

            # decoder state starts from the encoder's final (h, c) per layer
            if run_dec:
                h0d_ap = h0[:]
                h1d_ap = enc_view[:, :, S - 1]
                c0d_ap = c0[:]
                c1d_ap = c1[:]
            bounce = None
            for t in range(T if run_dec else 0):
                s_par = t % 2
                blk = t // 2
                # ---- attention ----
                engIn = sb.tile([H, NTE], f32, tag="engin")
                nc.vector.scalar_tensor_tensor(
                    out=engIn[:].rearrange("p (b s) -> p b s", b=BL),
                    in0=h1d_ap.unsqueeze(2).to_broadcast([H, BL, S]),
                    scalar=0.5,
                    in1=encprojT[:].rearrange("p (b s) -> p b s", b=BL),
                    op0=ALU.mult,
                    op1=ALU.add,
                )
                eng_ap = engIn[:]
                energy = sb.tile([H, NTE], f32, tag="energy")
                nc.scalar.activation(out=energy[:], in_=eng_ap, func=AF.Tanh)
                psS = pps.tile([1, NTE], f32, tag="ps")
                nc.tensor.matmul(psS[:], lhsT=c_attv[:, 0:1], rhs=energy[:], start=True, stop=True)
                eRow = sb.tile([1, NTE], f32, tag="eRow")
                nc.scalar.activation(out=eRow[:], in_=psS[:], func=AF.Exp)
                eB = sb.tile([H, NTE], f32, tag="eB")
                pbcast(eB, eRow[:], NTE)
                den = sb.tile([H, BL], f32, tag="den")
                nc.vector.reduce_sum(
                    out=den[:],
                    in_=eB[:].rearrange("p (b s) -> p b s", b=BL),
                    axis=AX.X,
                )
                rden = sb.tile([H, BL], f32, tag="rden")
                nc.vector.reciprocal(out=rden[:], in_=den[:])
                prod = sb.tile([H, NTE], f32, tag="prod")
                nc.vector.tensor_mul(out=prod[:], in0=encoutT[:], in1=eB[:])
                ctxU = sb.tile([H, BL], f32, tag="ctxU")
                nc.vector.reduce_sum(
                    out=ctxU[:],
                    in_=prod[:].rearrange("p (b s) -> p b s", b=BL),
                    axis=AX.X,
                )
                ctx2 = sb3.tile([H, BL], f32, tag="ctx2")
                nc.vector.tensor_mul(out=ctx2[:], in0=ctxU[:], in1=rden[:])

                # ---- decoder cells ----
                mats0 = [(c_dwih0cT, ctx2[:]), (c_dwhh0T, h0d_ap)]
                h0n = sb3.tile([H, BL], f32, tag="h0d")
                c0d = lstm_cell(
                    "d0", mats0, decihp[:, t * 4 * BL : (t + 1) * 4 * BL],
                    None, c0d_ap, h0n[:],
                )
                h0d_ap = h0n[:]
                c0d_ap = c0d[:]
                mats1 = [(c_dwih1T, h0d_ap), (c_dwhh1T, h1d_ap)]
                h1n = sb3.tile([H, BL], f32, tag="h1d")
                c1d = lstm_cell("d1", mats1, None, c_db1g, c1d_ap, h1n[:])
                h1d_ap = h1n[:]
                c1d_ap = c1d[:]

                # ---- gate (expert select) ----
                psG = pps.tile([1, BL], f32, tag="ps")
                nc.tensor.matmul(psG[:], lhsT=c_wd12[:, 0:1], rhs=h1d_ap, start=True, stop=False)
                nc.tensor.matmul(psG[:], lhsT=c_wd12[:, 1:2], rhs=ctx2[:], start=False, stop=True)
                sgn = sb.tile([1, BL], f32, tag="sgn")
                nc.scalar.activation(out=sgn[:], in_=psG[:], func=AF.Sign, bias=c_gdb[0:1, 0:1])
                m_row = sb.tile([1, BL], f32, tag="m_row")
                nc.vector.tensor_scalar(
                    out=m_row[:], in0=sgn[:], scalar1=1.0, scalar2=0.5,
                    op0=ALU.add, op1=ALU.mult,
                )

                # ---- payload store + gather + MoE ----
                if s_par == 0:
                    bounce = dr.tile([GSTEP, PAYR, BL], f32, tag="bounce")
                nc.sync.dma_start(out=bounce[s_par, 0:H, :], in_=h1d_ap)
                nc.sync.dma_start(out=bounce[s_par, H : 2 * H, :], in_=ctx2[:])
                nc.sync.dma_start(out=bounce[s_par, 2 * H : 2 * H + 1, :], in_=ones_row[:])
                nc.sync.dma_start(out=bounce[s_par, 2 * H + 1 : PAYR, :], in_=m_row[:])
                if s_par == 1 and DBG_PHASE >= 4:
                    gat = dr.tile([NCORES, GSTEP, PAYR, BL], f32, tag="gat")
                    if DBG_COLLECTIVE:
                        nc.gpsimd.collective_compute(
                            "AllGather",
                            ALU.bypass,
                            replica_groups=[list(range(NCORES))],
                            ins=[bounce.opt()],
                            outs=[gat.opt()],
                        )
                    else:
                        for cc in range(NCORES):
                            nc.sync.dma_start(out=gat[cc], in_=bounce[:])
                    moe_block(blk, gat)

    nc.compile()
    return nc


def _prep_host(inputs):
    """Build the per-core input maps (pure layout/shard prep)."""
    f = np.float32

    def ga(w):
        # [4H, D] pytorch gate order i,f,g,o -> i,f,o,g
        return np.concatenate([w[0:H], w[H : 2 * H], w[3 * H : 4 * H], w[2 * H : 3 * H]], axis=0)

    def gb(b):
        return np.concatenate([b[0:H], b[H : 2 * H], b[3 * H : 4 * H], b[2 * H : 3 * H]], axis=0)

    def bg_tile(b):
        return np.ascontiguousarray(gb(b).reshape(4, H).T).astype(f)

    emb = np.asarray(inputs["emb"], f)
    base = {
        "emb": np.ascontiguousarray(emb),
        "wih0T": np.ascontiguousarray(ga(np.asarray(inputs["enc_Wih0"], f)).T),
        "whh0T": np.ascontiguousarray(ga(np.asarray(inputs["enc_Whh0"], f)).T) * 0.5,
        "b0g": bg_tile(np.asarray(inputs["enc_b0"], f)),
        "wih1T": np.ascontiguousarray(ga(np.asarray(inputs["enc_Wih1"], f)).T) * 0.5,
        "whh1T": np.ascontiguousarray(ga(np.asarray(inputs["enc_Whh1"], f)).T) * 0.5,
        "b1g": bg_tile(np.asarray(inputs["enc_b1"], f)),
        "dwhh0T": np.ascontiguousarray(ga(np.asarray(inputs["dec_Whh0"], f)).T) * 0.5,
        "db0g": bg_tile(np.asarray(inputs["dec_b0"], f)),
        "dwih1T": np.ascontiguousarray(ga(np.asarray(inputs["dec_Wih1"], f)).T) * 0.5,
        "dwhh1T": np.ascontiguousarray(ga(np.asarray(inputs["dec_Whh1"], f)).T) * 0.5,
        "db1g": bg_tile(np.asarray(inputs["dec_b1"], f)),
        "attWT": np.ascontiguousarray(np.asarray(inputs["att_W"], f).T) * 0.5,
        "attb": np.asarray(inputs["att_b"], f).reshape(H, 1),
        "attv": np.asarray(inputs["att_v"], f).reshape(H, 1),
    }
    dwih0 = ga(np.asarray(inputs["dec_Wih0"], f))  # [512, E+H]
    dwih0T = np.ascontiguousarray(dwih0.T)         # [E+H, 512]
    base["dwih0xT"] = np.ascontiguousarray(dwih0T[0:E])
    base["dwih0cT"] = np.ascontiguousarray(dwih0T[E : E + H]) * 0.5

    gw = np.asarray(inputs["gate_W"], f)           # [2, 256]
    wd = (gw[0] - gw[1]) * 0.5
    base["wd12"] = np.ascontiguousarray(wd.reshape(2, H).T)
    gbv = np.asarray(inputs["gate_b"], f)
    base["gdb"] = np.array([[gbv[0] - gbv[1]]], f)

    expW = np.asarray(inputs["exp_W"], f)          # [2, V, 2H]
    expb = np.asarray(inputs["exp_b"], f)          # [2, V]
    src = np.asarray(inputs["src"], np.int32)
    trg = np.asarray(inputs["trg"], np.int32)

    in_maps = []
    for c in range(NCORES):
        m = dict(base)
        rows = slice(c * BL, (c + 1) * BL)
        m["src_idx"] = np.ascontiguousarray(src[rows].T).reshape(2, NTE // 2, 1)
        m["trg_idx"] = np.ascontiguousarray(trg[rows].T).reshape(2, NTD // 2, 1)
        vsl = slice(c * VS, (c + 1) * VS)
        W0 = expW[0, vsl]                          # [VS, 256]
        W1 = expW[1, vsl]
        w1T = W1.T * 0.5                           # [256, VS]
        wdT = (W0 - W1).T * 0.5
        m["w1a"] = np.ascontiguousarray(w1T[0:H])
        m["w1b"] = np.ascontiguousarray(w1T[H : 2 * H])
        m["wda"] = np.ascontiguousarray(wdT[0:H])
        m["wdb"] = np.ascontiguousarray(wdT[H : 2 * H])
        m["bias2"] = np.ascontiguousarray(
            np.stack([expb[1, vsl], expb[0, vsl] - expb[1, vsl]])
        )
        in_maps.append(m)
    return in_maps


last_results = None


def kernel(**inputs) -> np.ndarray:
    global last_results
    if "nc" not in _cache:
        _cache["nc"] = _build_program()
    nc = _cache["nc"]
    in_maps = _prep_host(inputs)
    trace = bool(os.environ.get("BASS_TRACE"))
    res = run_bass_kernel_spmd(
        nc, in_maps, core_ids=list(range(NCORES)), trace=trace
    )
    last_results = res
    # assemble: per-core out rows are (blk, c_src, s, b_local), cols = vocab shard
    parts = []
    for c in range(NCORES):
        o = res.results[c]["out"].reshape(NBLK, NCORES, GSTEP, BL, VS)
        parts.append(np.transpose(o, (1, 3, 0, 2, 4)).reshape(B, T, VS))
    return np.ascontiguousarray(np.concatenate(parts, axis=2))


# revision 35
# speedup vs baseline: 1.3849x; 1.3849x over previous
"""Trainium2 Bass kernel for EnhancedSeq2Seq (2-layer LSTM enc/dec + attention + 2-expert top-1 MoE vocab head).

Sharding: batch-parallel recurrent part (64/8 = 8 rows per core),
vocab-parallel MoE head (32000/8 = 4000 per core). Token features are
all-gathered across cores every 2 decoder steps and the MoE projection is
pipelined behind the decoder recurrence.

Scale conventions inside the device program:
  - h state tiles hold H = 2*h ("doubled h") so the sigmoid can be computed
    as a single tanh: sigmoid(x) = 0.5 + 0.5*tanh(x/2).  All weights that
    consume h (or doubled context CTX2 = 2*ctx) are pre-halved on the host.
  - encoutT holds doubled encoder outputs, att_WT is pre-halved.
  - MoE expert blend (top-1, K=1 => gate weight == 1):
      out = xf@W1 + (m*xf)@(W0-W1) + b1 + m*(b0-b1),  m = 1 if expert0 wins.
"""

import os
import sys

sys.path.insert(0, "/opt/trn_rl_repo")

import ml_dtypes
import numpy as np

import concourse.bass as bass
import concourse.mybir as mybir
import concourse.tile as tile
from concourse import bacc
from concourse.bass import IndirectOffsetOnAxis
from concourse.bass_utils import run_bass_kernel_spmd
from concourse.masks import make_identity

V, E, H = 32000, 64, 128
B, S, T = 64, 30, 20
NCORES = 8
BL = B // NCORES        # 8   local batch rows
VS = V // NCORES        # 4000 vocab shard
G4 = 4 * H              # 512
NTE = BL * S            # 240  encoder tokens / core
NTD = BL * T            # 160  decoder tokens / core
GSTEP = 4               # decoder steps per all-gather block
NGAT = T // GSTEP       # 5    all-gathers
NBLK = T // 2           # 10   128-token MoE blocks
TOKB = 2 * B            # 128  tokens per MoE block (all cores)
PAYR = 2 * H + 2        # 258  payload rows per step (H1, CTX2, ones, m)
VTILES = [(i * 512, min((i + 1) * 512, VS)) for i in range((VS + 511) // 512)]

f32 = mybir.dt.float32
f32r = mybir.dt.float32r
bf16 = mybir.dt.bfloat16
i32 = mybir.dt.int32
AF = mybir.ActivationFunctionType
ALU = mybir.AluOpType
AX = mybir.AxisListType

_cache = {}

# debug toggles for bisection
DBG_COLLECTIVE = True
DBG_PBCAST = True
DBG_GATHER = True
DBG_PHASE = 4  # 1=setup/embed/ihpre 2=+encoder 3=+decoder 4=+moe/collective


def _build_program():
    nc = bacc.Bacc("TRN2", target_bir_lowering=False, debug=False, num_devices=NCORES)

    # ---------------- I/O -------------------------------------------------
    din = {}

    def dram_in(name, shape, dtype=f32):
        din[name] = nc.dram_tensor(name, list(shape), dtype, kind="ExternalInput")
        return din[name]

    src_idx = dram_in("src_idx", [2, NTE // 2, 1], i32)
    trg_idx = dram_in("trg_idx", [2, NTD // 2, 1], i32)
    emb = dram_in("emb", [V, E])
    wih0T = dram_in("wih0T", [E, G4])
    whh0T = dram_in("whh0T", [H, G4])
    b0g = dram_in("b0g", [H, 4])
    wih1T = dram_in("wih1T", [H, G4])
    whh1T = dram_in("whh1T", [H, G4])
    b1g = dram_in("b1g", [H, 4])
    dwih0xT = dram_in("dwih0xT", [E, G4])
    dwih0cT = dram_in("dwih0cT", [H, G4])
    dwhh0T = dram_in("dwhh0T", [H, G4])
    db0g = dram_in("db0g", [H, 4])
    dwih1T = dram_in("dwih1T", [H, G4])
    dwhh1T = dram_in("dwhh1T", [H, G4])
    db1g = dram_in("db1g", [H, 4])
    attWT = dram_in("attWT", [H, H])
    attb = dram_in("attb", [H, 1])
    attv = dram_in("attv", [H, 1], bf16)
    wd12 = dram_in("wd12", [H, 2])
    gdb = dram_in("gdb", [1, 1])
    w1a = dram_in("w1a", [H, VS], f32r)
    w1b = dram_in("w1b", [H, VS], f32r)
    wda = dram_in("wda", [H, VS], f32r)
    wdb = dram_in("wdb", [H, VS], f32r)
    bias2 = dram_in("bias2", [2, VS], f32r)

    out = nc.dram_tensor("out", [NBLK * TOKB, VS], f32, kind="ExternalOutput")

    with tile.TileContext(nc) as tc:
        with (
            tc.tile_pool(name="wc", bufs=1) as wc,            # constants / persistents
            tc.tile_pool(name="sb", bufs=4) as sb,            # rotating work tiles
            tc.tile_pool(name="sb3", bufs=8) as sb3,          # recurrent state tiles
            tc.tile_pool(name="sbm", bufs=3) as sbm,          # MoE activation tiles
            tc.tile_pool(name="sbo", bufs=4) as sbo,          # MoE output staging
            tc.tile_pool(name="ppc", bufs=2, space="PSUM") as ppc,   # cell psums
            tc.tile_pool(name="ppe", bufs=2, space="PSUM") as ppe,   # [128,240] psums
            tc.tile_pool(name="pps", bufs=2, space="PSUM") as pps,   # small psums
            tc.tile_pool(name="ppo", bufs=2, space="PSUM") as ppo,   # MoE out psums
            tc.tile_pool(name="dr", bufs=4, space="DRAM") as dr,     # collective bufs
        ):
            # ---------------- constant loads ------------------------------
            idt = wc.tile([H, H], f32, tag="idt", name="idt")
            make_identity(nc, idt[:])

            # ---------------- embedding gather + transpose ----------------
            def embed(idx_dram, nchunks, chunk, xT):
                for i in range(nchunks):
                    isb = sb.tile([chunk, 1], i32, tag="isb")
                    nc.sync.dma_start(out=isb[:], in_=idx_dram[i])
                    gat = sb.tile([chunk, E], f32, tag="embg")
                    if DBG_GATHER:
                        nc.gpsimd.indirect_dma_start(
                            out=gat[:],
                            out_offset=None,
                            in_=emb[:, :],
                            in_offset=IndirectOffsetOnAxis(ap=isb[:, 0:1], axis=0),
                        )
                    else:
                        nc.sync.dma_start(out=gat[:], in_=emb[0:chunk, :])
                    pst = pps.tile([E, chunk], f32, tag="ps")
                    nc.tensor.transpose(pst[:], gat[:], idt[0:chunk, 0:chunk])
                    nc.scalar.copy(out=xT[:, i * chunk : (i + 1) * chunk], in_=pst[:])

            xT = wc.tile([E, NTE], f32, tag="xT", name="xT")
            xdT = wc.tile([E, NTD], f32, tag="xdT", name="xdT")
            embed(src_idx, 2, NTE // 2, xT)
            embed(trg_idx, 2, NTD // 2, xdT)

            _ct_count = [0]

            def const_tile(name, shape, dtype=f32, eng=None):
                t = wc.tile(list(shape), dtype, tag=name, name=name)
                if eng is None:
                    eng = nc.sync if _ct_count[0] % 2 == 0 else nc.scalar
                    _ct_count[0] += 1
                eng.dma_start(out=t[:], in_=din[name][:])
                return t

            c_wih0T = const_tile("wih0T", [E, G4])
            c_whh0T = const_tile("whh0T", [H, G4])
            c_b0g = const_tile("b0g", [H, 4])
            c_wih1T = const_tile("wih1T", [H, G4])
            c_whh1T = const_tile("whh1T", [H, G4])
            c_b1g = const_tile("b1g", [H, 4])
            c_dwih0xT = const_tile("dwih0xT", [E, G4])
            c_dwih0cT = const_tile("dwih0cT", [H, G4])
            c_dwhh0T = const_tile("dwhh0T", [H, G4])
            c_db0g = const_tile("db0g", [H, 4])
            c_dwih1T = const_tile("dwih1T", [H, G4])
            c_dwhh1T = const_tile("dwhh1T", [H, G4])
            c_db1g = const_tile("db1g", [H, 4])
            c_attWT = const_tile("attWT", [H, H])
            c_attb = const_tile("attb", [H, 1])
            c_attv = const_tile("attv", [H, 1], bf16)
            c_wd12 = const_tile("wd12", [H, 2])
            c_gdb = const_tile("gdb", [1, 1])

            # persistent activations
            ihpre0 = wc.tile([H, S * 4 * BL], f32, tag="ihpre0", name="ihpre0")
            decihp = wc.tile([H, T * 4 * BL], f32, tag="decihp", name="decihp")
            encoutT = wc.tile([H, NTE], f32, tag="encoutT", name="encoutT")   # (b, s) cols
            encprojT = wc.tile([H, NTE], f32, tag="encprojT", name="encprojT")
            ones_row = wc.tile([1, BL], f32, tag="ones_row", name="ones_row")
            nc.vector.memset(ones_row[:], 1.0)
            ones_l = wc.tile([1, H], bf16, tag="ones_l", name="ones_l")
            nc.vector.memset(ones_l[:], 1.0)

            def pbcast_psum(pool, tag, src_row, n):
                pb = pool.tile([H, n], f32, tag=tag)
                nc.tensor.matmul(pb[:], lhsT=ones_l[:], rhs=src_row, start=True, stop=True)
                return pb

            # ---------------- ih precomputes ------------------------------
            def ih_pre(dst, nt, wT, rhs, bg):
                dview = dst[:].rearrange("p (t g b) -> p t g b", t=nt, g=4, b=BL)
                for g in range(4):
                    ps = ppe.tile([H, nt * BL], f32, tag="pih")
                    nc.tensor.matmul(
                        ps[:], lhsT=wT[:, g * H : (g + 1) * H], rhs=rhs[:],
                        start=True, stop=True,
                    )
                    nc.scalar.activation(
                        out=dview[:, :, g, :],
                        in_=ps[:].rearrange("p (t b) -> p t b", t=nt, b=BL),
                        func=AF.Identity,
                        bias=bg[:, g : g + 1],
                    )

            ih_pre(ihpre0, S, c_wih0T, xT, c_b0g)
            ih_pre(decihp, T, c_dwih0xT, xdT, c_db0g)

            # ---------------- LSTM cell helper ----------------------------
            def bias_bcast(bg):
                return bg[:].unsqueeze(2).to_broadcast([H, 4, BL])

            def lstm_cell(tag, mats, z_pre_ap, z_bias, c_prev, h_out_ap):
                """mats: list of (lhsT_full[H,512], rhs_ap, start) matmul ops.
                z_pre_ap: SBUF precomputed (ih+bias) slice to add, or None.
                z_bias:   [H,4] bias tile to broadcast-add, or None.
                Returns new c tile. Writes H (=2h) into h_out_ap."""
                if mats:
                    ps = ppc.tile([H, 4 * BL], f32, tag="pz")
                    for g in range(4):
                        for j, (lhsT, rhs) in enumerate(mats):
                            nc.tensor.matmul(
                                ps[:, g * BL : (g + 1) * BL],
                                lhsT=lhsT[:, g * H : (g + 1) * H],
                                rhs=rhs,
                                start=(j == 0),
                                stop=(j == len(mats) - 1),
                            )
                    z = sb.tile([H, 4 * BL], f32, tag="z_" + tag)
                    if z_pre_ap is not None:
                        nc.vector.tensor_add(out=z[:], in0=ps[:], in1=z_pre_ap)
                    else:
                        nc.vector.tensor_tensor(
                            out=z[:].rearrange("p (g b) -> p g b", g=4),
                            in0=ps[:].rearrange("p (g b) -> p g b", g=4),
                            in1=bias_bcast(z_bias),
                            op=ALU.add,
                        )
                    z_ap = z[:]
                else:
                    z_ap = z_pre_ap
                # z's g-gate block is pre-doubled on the host, so one
                # tanh(0.5*z) covers sigmoid halves AND the true tanh(g).
                tio = sb.tile([H, 4 * BL], f32, tag="tio_" + tag)
                nc.scalar.activation(out=tio[:], in_=z_ap, func=AF.Tanh, scale=0.5)
                tg = tio[:, 3 * BL : 4 * BL]
                # cS carries 2*c ("doubled c"): avoids a separate 0.5 scale op
                bb = sb.tile([H, BL], f32, tag="bb_" + tag)
                nc.vector.scalar_tensor_tensor(
                    out=bb[:], in0=tio[:, 0:BL], scalar=1.0, in1=tg,
                    op0=ALU.add, op1=ALU.mult,
                )
                cS = sb3.tile([H, BL], f32, tag="c_" + tag)
                if c_prev is None:
                    nc.vector.tensor_copy(out=cS[:], in_=bb[:])
                else:
                    aa = sb.tile([H, BL], f32, tag="aa_" + tag)
                    nc.vector.scalar_tensor_tensor(
                        out=aa[:], in0=tio[:, BL : 2 * BL], scalar=1.0, in1=c_prev,
                        op0=ALU.add, op1=ALU.mult,
                    )
                    nc.vector.scalar_tensor_tensor(
                        out=cS[:], in0=aa[:], scalar=0.5, in1=bb[:],
                        op0=ALU.mult, op1=ALU.add,
                    )
                tch = sb.tile([H, BL], f32, tag="tc_" + tag)
                nc.scalar.activation(out=tch[:], in_=cS[:], func=AF.Tanh, scale=0.5)
                nc.vector.scalar_tensor_tensor(
                    out=h_out_ap, in0=tio[:, 2 * BL : 3 * BL], scalar=1.0, in1=tch[:],
                    op0=ALU.add, op1=ALU.mult,
                )
                return cS

            # ---------------- encoder ------------------------------------
            enc_view = encoutT[:].rearrange("p (b s) -> p b s", b=BL, s=S)
            if DBG_PHASE < 2:
                S_eff = 0
            else:
                S_eff = S
            h0 = c0 = c1 = None
            h1_ap = None
            for t in range(S_eff):
                mats0 = [] if t == 0 else [(c_whh0T, h0[:])]
                h0n = sb3.tile([H, BL], f32, tag="h0e")
                c0 = lstm_cell(
                    "e0", mats0, ihpre0[:, t * 4 * BL : (t + 1) * 4 * BL],
                    None, c0 if c0 is None else c0[:], h0n[:],
                )
                h0 = h0n
                mats1 = [(c_wih1T, h0[:])]
                if t > 0:
                    mats1.append((c_whh1T, h1_ap))
                h1_ap = enc_view[:, :, t]
                c1 = lstm_cell("e1", mats1, None, c_b1g, c1 if c1 is None else c1[:], h1_ap)

            # big MoE weights: emitted after the encoder so their DMA
            # bandwidth doesn't contend with the setup-critical loads
            c_w1a = const_tile("w1a", [H, VS], f32r, eng=nc.scalar)
            c_w1b = const_tile("w1b", [H, VS], f32r, eng=nc.scalar)
            c_wda = const_tile("wda", [H, VS], f32r, eng=nc.scalar)
            c_wdb = const_tile("wdb", [H, VS], f32r, eng=nc.scalar)
            c_bias2 = const_tile("bias2", [2, VS], f32r, eng=nc.scalar)

            # ---------------- encoder projection --------------------------
            run_dec = DBG_PHASE >= 3 and S_eff == S
            if S_eff == S:
                psP = ppe.tile([H, NTE], f32, tag="pih")
                nc.tensor.matmul(psP[:], lhsT=c_attWT[:], rhs=encoutT[:], start=True, stop=True)
                nc.scalar.activation(
                    out=encprojT[:], in_=psP[:], func=AF.Identity, bias=c_attb[:, 0:1]
                )

            # ---------------- decoder + MoE -------------------------------
            def moe_block(blk, gat, s0):
                gv = gat[:, s0 : s0 + 2, :, :]
                xf1 = sbm.tile([H, TOKB], f32r, tag="xf1")
                xf2 = sbm.tile([H, TOKB], f32r, tag="xf2")
                b2T = sbm.tile([2, TOKB], f32r, tag="b2T")
                for dst, r0, r1 in ((xf1, 0, H), (xf2, H, 2 * H), (b2T, 2 * H, PAYR)):
                    for sj in range(2):
                        nc.scalar.dma_start(
                            out=dst[:].rearrange("p (c s b) -> p c s b", c=NCORES, s=2, b=BL)[:, :, sj, :],
                            in_=gv[:, sj, r0:r1, :].rearrange("c r b -> r c b").bitcast(f32r),
                        )
                mRow = sbm.tile([1, TOKB], f32, tag="mRow")
                for sj in range(2):
                    nc.scalar.dma_start(
                        out=mRow[:].rearrange("p (c s b) -> p c s b", c=NCORES, s=2, b=BL)[:, :, sj, :],
                        in_=gv[:, sj, PAYR - 1 : PAYR, :].rearrange("c r b -> r c b"),
                    )
                mB = sbm.tile([H, TOKB], f32, tag="mB")
                nc.gpsimd.partition_broadcast(mB[:], mRow[:])
                x01 = sbm.tile([H, TOKB], f32r, tag="x01")
                x02 = sbm.tile([H, TOKB], f32r, tag="x02")
                nc.vector.tensor_mul(out=x01[:], in0=xf1[:], in1=mB[:])
                nc.vector.tensor_mul(out=x02[:], in0=xf2[:], in1=mB[:])
                for nv, (lo, hi) in enumerate(VTILES):
                    w = hi - lo
                    po = ppo.tile([TOKB, 512], f32, tag="po")
                    sl = slice(lo, hi)
                    mms = [
                        (xf1, c_w1a), (xf2, c_w1b), (x01, c_wda), (x02, c_wdb), (b2T, c_bias2),
                    ]
                    for j, (lt, rt) in enumerate(mms):
                        nc.tensor.matmul(
                            po[:, 0:w],
                            lhsT=lt[:],
                            rhs=rt[:, sl],
                            start=(j == 0),
                            stop=(j == len(mms) - 1),
                        )
                    st = sbo.tile([TOKB, 512], f32, tag="st")
                    if nv % 2 == 0:
                        nc.scalar.copy(out=st[:, 0:w], in_=po[:, 0:w])
                    else:
                        nc.vector.tensor_copy(out=st[:, 0:w], in_=po[:, 0:w])
                    nc.scalar.dma_start(
                        out=out[blk * TOKB : (blk + 1) * TOKB, sl], in_=st[:, 0:w]
                    )

            # decoder state starts from the encoder's final (h, c) per layer
            if run_dec:
                h0d_ap = h0[:]
                h1d_ap = enc_view[:, :, S - 1]
                c0d_ap = c0[:]
                c1d_ap = c1[:]
            bounce = None
            moe_pend = []
            for t in range(T if run_dec else 0):
                s_par = t % 2
                blk = t // 2
                # ---- attention ----
                engIn = sb.tile([H, NTE], f32, tag="engin")
                nc.vector.scalar_tensor_tensor(
                    out=engIn[:].rearrange("p (b s) -> p b s", b=BL),
                    in0=h1d_ap.unsqueeze(2).to_broadcast([H, BL, S]),
                    scalar=0.5,
                    in1=encprojT[:].rearrange("p (b s) -> p b s", b=BL),
                    op0=ALU.mult,
                    op1=ALU.add,
                )
                eng_ap = engIn[:]
                energy = sb.tile([H, NTE], bf16, tag="energy")
                nc.scalar.activation(out=energy[:], in_=eng_ap, func=AF.Tanh)
                psS = pps.tile([1, NTE], f32, tag="ps")
                nc.tensor.matmul(psS[:], lhsT=c_attv[:, 0:1], rhs=energy[:], start=True, stop=True)
                eRow = sb.tile([1, NTE], bf16, tag="eRow")
                nc.scalar.activation(out=eRow[:], in_=psS[:], func=AF.Exp)
                eB = pbcast_psum(ppe, "pih", eRow[:], NTE)
                den = sb.tile([H, BL], f32, tag="den")
                nc.vector.reduce_sum(
                    out=den[:],
                    in_=eB[:].rearrange("p (b s) -> p b s", b=BL),
                    axis=AX.X,
                )
                rden = sb.tile([H, BL], f32, tag="rden")
                nc.vector.reciprocal(out=rden[:], in_=den[:])
                prod = sb.tile([H, NTE], f32, tag="prod")
                nc.vector.tensor_mul(out=prod[:], in0=encoutT[:], in1=eB[:])
                ctxU = sb.tile([H, BL], f32, tag="ctxU")
                nc.vector.reduce_sum(
                    out=ctxU[:],
                    in_=prod[:].rearrange("p (b s) -> p b s", b=BL),
                    axis=AX.X,
                )
                ctx2 = sb3.tile([H, BL], f32, tag="ctx2")
                nc.vector.tensor_mul(out=ctx2[:], in0=ctxU[:], in1=rden[:])

                # ---- decoder cells ----
                mats0 = [(c_dwih0cT, ctx2[:]), (c_dwhh0T, h0d_ap)]
                h0n = sb3.tile([H, BL], f32, tag="h0d")
                c0d = lstm_cell(
                    "d0", mats0, decihp[:, t * 4 * BL : (t + 1) * 4 * BL],
                    None, c0d_ap, h0n[:],
                )
                h0d_ap = h0n[:]
                c0d_ap = c0d[:]
                mats1 = [(c_dwih1T, h0d_ap), (c_dwhh1T, h1d_ap)]
                h1n = sb3.tile([H, BL], f32, tag="h1d")
                c1d = lstm_cell("d1", mats1, None, c_db1g, c1d_ap, h1n[:])
                h1d_ap = h1n[:]
                c1d_ap = c1d[:]

                # ---- gate (expert select) ----
                psG = pps.tile([1, BL], f32, tag="ps")
                nc.tensor.matmul(psG[:], lhsT=c_wd12[:, 0:1], rhs=h1d_ap, start=True, stop=False)
                nc.tensor.matmul(psG[:], lhsT=c_wd12[:, 1:2], rhs=ctx2[:], start=False, stop=True)
                sgn = sb.tile([1, BL], f32, tag="sgn")
                nc.scalar.activation(out=sgn[:], in_=psG[:], func=AF.Sign, bias=c_gdb[0:1, 0:1])
                m_row = sb.tile([1, BL], f32, tag="m_row")
                nc.vector.tensor_scalar(
                    out=m_row[:], in0=sgn[:], scalar1=1.0, scalar2=0.5,
                    op0=ALU.add, op1=ALU.mult,
                )

                # ---- payload store + gather + MoE ----
                s_par = t % GSTEP
                blk = t // GSTEP
                if s_par == 0:
                    bounce = dr.tile([GSTEP, PAYR, BL], f32, tag="bounce")
                nc.sync.dma_start(out=bounce[s_par, 0:H, :], in_=h1d_ap)
                nc.sync.dma_start(out=bounce[s_par, H : 2 * H, :], in_=ctx2[:])
                nc.sync.dma_start(out=bounce[s_par, 2 * H : 2 * H + 1, :], in_=ones_row[:])
                nc.sync.dma_start(out=bounce[s_par, 2 * H + 1 : PAYR, :], in_=m_row[:])
                if s_par == GSTEP - 1 and DBG_PHASE >= 4:
                    gat = dr.tile([NCORES, GSTEP, PAYR, BL], f32, tag="gat")
                    if DBG_COLLECTIVE:
                        nc.gpsimd.collective_compute(
                            "AllGather",
                            ALU.bypass,
                            replica_groups=[list(range(NCORES))],
                            ins=[bounce.opt()],
                            outs=[gat.opt()],
                        )
                    else:
                        for cc in range(NCORES):
                            nc.sync.dma_start(out=gat[cc], in_=bounce[:])
                    for args in moe_pend:
                        moe_block(*args)
                    moe_pend = [(2 * blk, gat, 0), (2 * blk + 1, gat, 2)]
            for args in moe_pend:
                moe_block(*args)

    nc.compile()
    return nc


def _prep_host(inputs):
    """Build the per-core input maps (pure layout/shard prep)."""
    f = np.float32

    def dblw(wT):
        # double the g-gate column block so one tanh(0.5*z) serves all gates
        wT = wT.copy()
        wT[:, 3 * H : 4 * H] *= 2.0
        return wT

    def dblb(bg):
        bg = bg.copy()
        bg[:, 3] *= 2.0
        return bg

    def ga(w):
        # [4H, D] pytorch gate order i,f,g,o -> i,f,o,g
        return np.concatenate([w[0:H], w[H : 2 * H], w[3 * H : 4 * H], w[2 * H : 3 * H]], axis=0)

    def gb(b):
        return np.concatenate([b[0:H], b[H : 2 * H], b[3 * H : 4 * H], b[2 * H : 3 * H]], axis=0)

    def bg_tile(b):
        return np.ascontiguousarray(gb(b).reshape(4, H).T).astype(f)

    emb = np.asarray(inputs["emb"], f)
    base = {
        "emb": np.ascontiguousarray(emb),
        "wih0T": dblw(np.ascontiguousarray(ga(np.asarray(inputs["enc_Wih0"], f)).T)),
        "whh0T": dblw(np.ascontiguousarray(ga(np.asarray(inputs["enc_Whh0"], f)).T) * 0.5),
        "b0g": dblb(bg_tile(np.asarray(inputs["enc_b0"], f))),
        "wih1T": dblw(np.ascontiguousarray(ga(np.asarray(inputs["enc_Wih1"], f)).T) * 0.5),
        "whh1T": dblw(np.ascontiguousarray(ga(np.asarray(inputs["enc_Whh1"], f)).T) * 0.5),
        "b1g": dblb(bg_tile(np.asarray(inputs["enc_b1"], f))),
        "dwhh0T": dblw(np.ascontiguousarray(ga(np.asarray(inputs["dec_Whh0"], f)).T) * 0.5),
        "db0g": dblb(bg_tile(np.asarray(inputs["dec_b0"], f))),
        "dwih1T": dblw(np.ascontiguousarray(ga(np.asarray(inputs["dec_Wih1"], f)).T) * 0.5),
        "dwhh1T": dblw(np.ascontiguousarray(ga(np.asarray(inputs["dec_Whh1"], f)).T) * 0.5),
        "db1g": dblb(bg_tile(np.asarray(inputs["dec_b1"], f))),
        "attWT": np.ascontiguousarray(np.asarray(inputs["att_W"], f).T) * 0.5,
        "attb": np.asarray(inputs["att_b"], f).reshape(H, 1),
        "attv": np.asarray(inputs["att_v"], f).reshape(H, 1).astype(ml_dtypes.bfloat16),
    }
    dwih0 = ga(np.asarray(inputs["dec_Wih0"], f))  # [512, E+H]
    dwih0T = np.ascontiguousarray(dwih0.T)         # [E+H, 512]
    base["dwih0xT"] = dblw(np.ascontiguousarray(dwih0T[0:E]))
    base["dwih0cT"] = dblw(np.ascontiguousarray(dwih0T[E : E + H]) * 0.5)

    gw = np.asarray(inputs["gate_W"], f)           # [2, 256]
    wd = (gw[0] - gw[1]) * 0.5
    base["wd12"] = np.ascontiguousarray(wd.reshape(2, H).T)
    gbv = np.asarray(inputs["gate_b"], f)
    base["gdb"] = np.array([[gbv[0] - gbv[1]]], f)

    expW = np.asarray(inputs["exp_W"], f)          # [2, V, 2H]
    expb = np.asarray(inputs["exp_b"], f)          # [2, V]
    src = np.asarray(inputs["src"], np.int32)
    trg = np.asarray(inputs["trg"], np.int32)

    in_maps = []
    for c in range(NCORES):
        m = dict(base)
        rows = slice(c * BL, (c + 1) * BL)
        m["src_idx"] = np.ascontiguousarray(src[rows].T).reshape(2, NTE // 2, 1)
        m["trg_idx"] = np.ascontiguousarray(trg[rows].T).reshape(2, NTD // 2, 1)
        vsl = slice(c * VS, (c + 1) * VS)
        W0 = expW[0, vsl]                          # [VS, 256]
        W1 = expW[1, vsl]
        w1T = W1.T * 0.5                           # [256, VS]
        wdT = (W0 - W1).T * 0.5
        m["w1a"] = np.ascontiguousarray(w1T[0:H])
        m["w1b"] = np.ascontiguousarray(w1T[H : 2 * H])
        m["wda"] = np.ascontiguousarray(wdT[0:H])
        m["wdb"] = np.ascontiguousarray(wdT[H : 2 * H])
        m["bias2"] = np.ascontiguousarray(
            np.stack([expb[1, vsl], expb[0, vsl] - expb[1, vsl]])
        )
        in_maps.append(m)
    return in_maps


last_results = None


def kernel(**inputs) -> np.ndarray:
    global last_results
    if "nc" not in _cache:
        _cache["nc"] = _build_program()
    nc = _cache["nc"]
    in_maps = _prep_host(inputs)
    trace = bool(os.environ.get("BASS_TRACE"))
    res = run_bass_kernel_spmd(
        nc, in_maps, core_ids=list(range(NCORES)), trace=trace
    )
    last_results = res
    # assemble: per-core out rows are (blk, c_src, s, b_local), cols = vocab shard
    parts = []
    for c in range(NCORES):
        o = res.results[c]["out"].reshape(NBLK, NCORES, 2, BL, VS)
        parts.append(np.transpose(o, (1, 3, 0, 2, 4)).reshape(B, T, VS))
    return np.ascontiguousarray(np.concatenate(parts, axis=2))
